# revision 21
# baseline (speedup 1.0000x reference)
"""Causal dot-product attention (s=2048, b=4, h=16, d=128) on 8 TRN2 NeuronCores.

Sharding: batch*heads (64 pairs) split across 8 cores -> 8 (b,h) pairs per core.
Core c handles b = c // 2, heads h in [(c%2)*8, (c%2)*8 + 8).

Per-core kernel (Bass/Tile), per head:
  S^T[sk, sq] = K^T_j(stationary) . Q^T(moving)   (fp16 in, fp32 PSUM out)
  E = exp(S^T * 1/sqrt(d))  (ACT, fp16 out)
  causal: skip sk>sq blocks/columns; triangular fp16 mask multiply on the
          diagonal 128-wide subtile only (DVE)
  ctx[sq, 0:128] + rowsum[sq] (col 128) = sum_j E_j^T(stationary) . [V_j | 1]
  out = ctx * (1/rowsum)     (DVE reciprocal + per-partition scalar multiply)

The kernel is ACT-limited: exp covers ~17.4k psum columns per head at
1 col/cycle @1.2GHz and each ACTIVATE carries a ~308-cycle fixed overhead,
so exp groups are as wide as PSUM allows: s_ps is 2 x [128,1536] (3 banks
each, 6 of 8 banks) -> 13 ACTIVATEs per head (vs 20 for 1024-wide groups):
  per i5 (512-row sq block): full j-tiles exp'd in groups of <=3 (1536
  cols); the 4 diagonal j-tiles packed hole-free into one 1280-col group
  laid out [t0@0 | t3@512 | t1@640 | t2@1024] (each trimmed to its
  causally-live columns, every matmul write within one PSUM bank; t1 rides
  t3's start=True bank clear with start=False overwrite semantics).

That leaves only 2 banks for the ctx accumulators (single-buffered, one
bank per tt pair), so each i5's epilogue is emitted inside its diagonal
group as soon as the corresponding ctx bank closes (tt0/1 after the t=1
chain, tt2/3 at group end); the diag masks are queued on DVE before the
epilogue so the strict FIFO never delays PV.  The next i5 reuses the bank
roughly two group-periods later, which the epilogue comfortably beats.

Scheduling: two-period software pipelining.  PE program order is
  QK(g), PV(g-2), QK(g+1), PV(g-1), ...
with exp(g) on the scalar queue right behind its QK.  The ACT chain then
only ever waits [exp(g-2) -> QK(g)] (shorter than one exp duration), so
the exp engine runs back-to-back at ~99% of its busy floor; PV(g) runs a
full period after exp(g), entirely off the critical path, which also hides
the diag-mask DVE latency and the ctx-bank WAR dependencies at i5 seams.

Startup: dummy 128-col PE matmuls at t=0 warm the HAM clock gate during
the DMA prologue (otherwise early matmuls run at 1.2 instead of 2.4 GHz);
a tiny dummy exp preloads the ACT exp spline table; head 0's qk DMA is
streamed in compute order so the first QK group starts ~1.5us in.

Host-side layout prep: Q and K are transposed to [head, d, s], concatenated,
and cast to fp16.  V is cast to fp16 with the softmax-denominator
ones-column baked in.
"""

import sys

if "/opt/trn_rl_repo" not in sys.path:
    sys.path.insert(0, "/opt/trn_rl_repo")

import numpy as np

import concourse.bacc as bacc
import concourse.bass as bass
import concourse.mybir as mybir
import concourse.tile as tile
from concourse.bass_utils import run_bass_kernel_spmd

S, B, H, D = 2048, 4, 16, 128
N_CORES = 8
HPC = (B * H) // N_CORES  # heads per core = 8
SCALE = 1.0 / float(np.sqrt(128.0))

SQ_BLK = 512  # sq block width per i5 row
N_I = S // SQ_BLK  # 4 sq blocks per head
N_SK = S // 128  # 16 sk tiles per head
VW = 129  # V tile width incl. ones column
N_WARM_MM = 22  # HAM warm-up dummy matmuls (128 cols each; they fill the
# otherwise-idle PE window while head 0's first qk DMA is in flight, so the
# clock gate is at 8/8 when real matmuls start)


def head_groups():
    """13 (kind, i5, tiles) groups; tiles = [(j, off, width, qc0), ...].

    off = column offset in the [128,1536] s_ps / e_sb tile, width = live
    sq columns, qc0 = first live sq column within the 512-wide i5 block.
    """
    gs = []
    full_splits = {0: [], 1: [2, 2], 2: [3, 3, 2], 3: [3, 3, 3, 3]}
    for i5 in range(N_I):
        pos = 0
        for k in full_splits[i5]:
            tiles = [(pos + q, q * 512, 512, 0) for q in range(k)]
            pos += k
            gs.append(("F", i5, tiles))
        # diagonal tiles t=0..3 (j = 4*i5+t), packed hole-free:
        # [t0(512)@0 | t3(128)@512 | t1(384)@640 | t2(256)@1024]
        tiles = [
            (4 * i5 + t, off, 512 - 128 * t, 128 * t)
            for t, off in [(0, 0), (3, 512), (1, 640), (2, 1024)]
        ]
        gs.append(("D", i5, tiles))
    return gs


GROUPS = head_groups()
# head 0 ramp: its first diagonal group is split so the very first exp only
# needs Q block 0 + K tile j0 (the first two DMAs): a 512-col starter [t0]
# and a 768-col remainder [t3@0 | t1@128 | t2@512] (t1 rides t3's bank
# clear). One extra ACTIVATE, ~1us earlier pipeline start.
GROUPS_H0 = [
    ("Dt0", 0, [(0, 0, 512, 0)]),
    ("Dr", 0, [(3, 0, 128, 384), (1, 128, 384, 128), (2, 512, 256, 256)]),
] + GROUPS[1:]
G_WIDTH = {"D": 1280, "Dt0": 512, "Dr": 768}


def build_nc():
    nc = bacc.Bacc()
    qk = nc.dram_tensor("qk", [HPC, D, 2 * S], mybir.dt.float16, kind="ExternalInput")
    v = nc.dram_tensor("v", [HPC, N_SK, 128, VW], mybir.dt.float16, kind="ExternalInput")
    out = nc.dram_tensor("out", [S, HPC * D], mybir.dt.float32, kind="ExternalOutput")

    with tile.TileContext(nc) as tc:
        with (
            tc.tile_pool(name="const", bufs=1) as constp,
            tc.tile_pool(name="qkp", bufs=2) as qkp,
            tc.tile_pool(name="vp", bufs=3) as vpool,
            tc.tile_pool(name="e", bufs=6) as ep,
            tc.tile_pool(name="stage", bufs=3) as stagep,
            tc.tile_pool(name="rec", bufs=8) as recp,
            tc.tile_pool(name="em", bufs=8) as emp,
            tc.tile_pool(name="ps_s", bufs=2, space="PSUM") as ps_s,
            tc.tile_pool(name="ps_c", bufs=1, space="PSUM") as ps_c,
        ):
            # tri[r, c] = 1.0 if c >= r else 0.0 (fp16) - diagonal-subtile mask
            tri = constp.tile([128, 128], mybir.dt.float16)
            nc.gpsimd.memset(tri[:], 1.0)
            nc.gpsimd.affine_select(
                out=tri[:],
                in_=tri[:],
                compare_op=mybir.AluOpType.is_ge,
                fill=0.0,
                base=0,
                pattern=[[1, 128]],
                channel_multiplier=-1,
            )
            # tiny dummy exp: triggers the one-time ~2.7us ACT table load
            # during the DMA prologue instead of before the first real exp
            warm = constp.tile([1, 8], mybir.dt.float32, name="warm")
            nc.vector.memset(warm[:], 0.0)
            nc.scalar.activation(
                warm[:],
                warm[:],
                mybir.ActivationFunctionType.Exp,
                scale=SCALE,
            )
            # HAM warm-up: keep PE busy through the DMA prologue so the
            # clock gate reaches 8/8 by the time real matmuls start
            warm_sb = constp.tile([128, 128], mybir.dt.float16, name="warm_sb")
            nc.vector.memset(warm_sb[:], 0.0)
            dum_ps = ps_s.tile([128, 128], mybir.dt.float32, tag="s", name="dum")
            for _ in range(N_WARM_MM):
                nc.tensor.matmul(
                    dum_ps[:],
                    warm_sb[:],
                    warm_sb[:],
                    start=True,
                    stop=True,
                    skip_group_check=True,
                )

            started_heads = set()
            started_i5 = set()
            vdummy_done = set()
            ctx_holder = {}
            staged_holder = {}
            qk_holder = {}
            v_holder = {}

            def start_head(hh):
                qk_sb = qkp.tile([128, 2 * S], mybir.dt.float16, tag="qk", name="qk_sb")
                qk_holder[hh] = qk_sb
                v_sb = vpool.tile([128, N_SK * VW], mybir.dt.float16, tag="v", name="v_sb")
                v_holder[hh] = v_sb
                qk4 = qk_sb.rearrange("p (b c) -> p b c", c=SQ_BLK)
                qk4s = qk[hh, :, :].rearrange("p (b c) -> p b c", c=SQ_BLK)
                v3 = v_sb.rearrange("p (j e) -> p j e", e=VW)
                v3s = v[hh, :, :, :].rearrange("j p e -> p j e")
                if hh == 0:
                    # head 0 has no prefetch window: stream qk in compute
                    # order, one block ahead of the matching v tiles; the
                    # first Q/K blocks go as two DMAs to ride parallel queues
                    nc.sync.dma_start(out=qk4[:, 0, :], in_=qk4s[:, 0, :])
                    nc.sync.dma_start(
                        out=qk_sb[:, S : S + 128], in_=qk[hh, :, S : S + 128]
                    )
                    nc.sync.dma_start(
                        out=qk_sb[:, S + 128 : S + 512],
                        in_=qk[hh, :, S + 128 : S + 512],
                    )
                    for b in range(1, N_I):
                        nc.sync.dma_start(
                            out=qk4[:, b :: N_I, :], in_=qk4s[:, b :: N_I, :]
                        )
                        nc.sync.dma_start(
                            out=v3[:, 4 * (b - 1) : 4 * b, :],
                            in_=v3s[:, 4 * (b - 1) : 4 * b, :],
                        )
                    nc.sync.dma_start(
                        out=v3[:, 4 * (N_I - 1) :, :], in_=v3s[:, 4 * (N_I - 1) :, :]
                    )
                else:
                    # later heads are fully prefetched during the previous head
                    nc.sync.dma_start(out=qk_sb[:], in_=qk[hh, :, :])
                    nc.sync.dma_start(out=v3, in_=v3s)
                staged_holder[hh] = stagep.tile(
                    [128, N_SK * D], mybir.dt.float32, tag="o", name="staged"
                )

            def start_i5(hh, i5):
                ctx_ab = [
                    ps_c.tile(
                        [128, 2 * VW], mybir.dt.float32, tag=f"ctx{t}", name=f"ctx{t}"
                    )
                    for t in range(2)
                ]
                ctx_holder[(hh, i5)] = (
                    ctx_ab,
                    [
                        ctx_ab[tt // 2][:, (tt % 2) * VW : (tt % 2 + 1) * VW]
                        for tt in range(4)
                    ],
                )

            def emit_qk(hh, grp):
                kind, i5, tiles = grp
                if hh not in started_heads:
                    start_head(hh)
                    started_heads.add(hh)
                if hh + 1 < HPC and hh + 1 not in started_heads:
                    # issue the next head's DMAs a full head ahead
                    start_head(hh + 1)
                    started_heads.add(hh + 1)
                if (hh, i5) not in started_i5:
                    start_i5(hh, i5)
                    started_i5.add((hh, i5))
                qk_sb = qk_holder[hh]
                s_ps = ps_s.tile([128, 1536], mybir.dt.float32, tag="s", name="s_ps")
                seen_banks = set()
                for j, off, width, qc0 in tiles:
                    bank = off // 512
                    first = bank not in seen_banks
                    seen_banks.add(bank)
                    nc.tensor.matmul(
                        s_ps[:, off : off + width],
                        qk_sb[:, S + j * 128 : S + (j + 1) * 128],
                        qk_sb[:, i5 * SQ_BLK + qc0 : (i5 + 1) * SQ_BLK],
                        start=first,
                        stop=True,
                    )
                return s_ps

            def epi_pair(hh, i5, ctx_ab, ctx_t, b):
                # drain ctx bank b (tt = 2b, 2b+1): one strided reciprocal
                # over both rowsum columns, then per-tt normalize + DMA out
                staged = staged_holder[hh]
                tts = (2 * b, 2 * b + 1)
                rec = recp.tile([128, 2], mybir.dt.float32, tag="rec", name="rec")
                z2 = ctx_ab[b].rearrange("p (t e) -> p t e", e=VW)[:, :, 128]
                nc.vector.reciprocal(rec[:], z2)
                for tt in tts:
                    nc.vector.tensor_scalar_mul(
                        staged[:, (i5 * 4 + tt) * D : (i5 * 4 + tt + 1) * D],
                        ctx_t[tt][:, 0:128],
                        rec[:, tt % 2 : tt % 2 + 1],
                    )
                r0 = i5 * SQ_BLK + tts[0] * 128
                nc.sync.dma_start(
                    out=out[r0 : r0 + 256, hh * D : (hh + 1) * D].rearrange(
                        "(i p) d -> p i d", p=128
                    ),
                    in_=staged.rearrange("p (i d) -> p i d", d=D)[
                        :, i5 * 4 + tts[0] : i5 * 4 + tts[1] + 1, :
                    ],
                )

            def emit_exp(hh, grp, s_ps):
                kind, i5, tiles = grp
                width_total = G_WIDTH.get(kind) or 512 * len(tiles)
                e_sb = ep.tile([128, 1536], mybir.dt.float16, tag="e", name="e_sb")
                nc.scalar.activation(
                    e_sb[:, 0:width_total],
                    s_ps[:, 0:width_total],
                    mybir.ActivationFunctionType.Exp,
                    scale=SCALE,
                )
                return e_sb

            def emit_pv(hh, grp, e_sb):
                kind, i5, tiles = grp
                v_sb = v_holder[hh]
                ctx_ab, ctx_t = ctx_holder[(hh, i5)]
                if hh not in vdummy_done:
                    # absorb the v-DMA wait on PE right before the head's
                    # first PV matmul (scribbles on ctx, which the j=0
                    # start=True matmul then resets)
                    vdummy_done.add(hh)
                    nc.tensor.matmul(
                        ctx_t[0][0:1, 0:8],
                        v_sb[:, 0:1],
                        v_sb[:, 0:8],
                        start=True,
                        stop=True,
                        skip_group_check=True,
                    )
                em_tiles = {}
                if kind != "F":
                    # all 4 masked diagonal subtiles enter the DVE FIFO ahead
                    # of this group's epilogue ops so PV never waits on them
                    for j, off, width, qc0 in sorted(tiles):
                        t = j - 4 * i5
                        em = emp.tile([128, 128], mybir.dt.float16, tag="em", name="em")
                        nc.vector.tensor_mul(em[:], e_sb[:, off : off + 128], tri[:])
                        em_tiles[t] = em
                # ascending-j emission keeps j=0's bank-clearing start=True
                # matmuls ahead of every other writer of the same psum bank
                for j, off, width, qc0 in sorted(tiles):
                    t = (j - 4 * i5) if kind != "F" else -1
                    t0 = max(t, 0)
                    for tt in range(t0, 4):
                        lhs = (
                            em_tiles[t][:]
                            if (t >= 0 and tt == t)
                            else e_sb[
                                :, off + (tt - t0) * 128 : off + (tt - t0 + 1) * 128
                            ]
                        )
                        nc.tensor.matmul(
                            ctx_t[tt][:],
                            lhs,
                            v_sb[:, j * VW : (j + 1) * VW],
                            start=(j == 0 and tt % 2 == 0),
                            stop=(kind != "F" and j - 4 * i5 == tt),
                            skip_group_check=True,
                        )
                    if kind in ("D", "Dr") and t == 1:
                        # tt0/tt1 are fully accumulated: drain their bank
                        # while PE runs the t=2/t=3 chains
                        epi_pair(hh, i5, ctx_ab, ctx_t, 0)
                if kind in ("D", "Dr"):
                    epi_pair(hh, i5, ctx_ab, ctx_t, 1)

            groups = [
                (hh, grp)
                for hh in range(HPC)
                for grp in (GROUPS_H0 if hh == 0 else GROUPS)
            ]
            # two-period software pipelining: PE program order is
            #   QK(g), PV(g-2), QK(g+1), PV(g-1), ...
            # exp(g) follows its QK immediately on the scalar queue, so the
            # ACT chain only ever waits [exp(g-2) -> QK(g)] (shorter than one
            # exp) and runs back-to-back; PV(g) runs a full period after
            # exp(g), off the critical path, with its diag masks and ctx-bank
            # epilogues long settled before their results are needed again.
            pend = []
            for g in groups:
                s_ps = emit_qk(*g)
                e_sb = emit_exp(*g, s_ps)
                pend.append((g, e_sb))
                if len(pend) > 2:
                    pg, pe_sb = pend.pop(0)
                    emit_pv(*pg, pe_sb)
            for pg, pe_sb in pend:
                emit_pv(*pg, pe_sb)
    nc.compile()
    return nc


_NC_CACHE = None


def _get_nc():
    global _NC_CACHE
    if _NC_CACHE is None:
        _NC_CACHE = build_nc()
    return _NC_CACHE


def _make_in_maps(query_layer, key_layer, value_layer):
    q = np.asarray(query_layer)
    k = np.asarray(key_layer)
    v = np.asarray(value_layer)
    in_maps = []
    for c in range(N_CORES):
        b = c // 2
        h0 = (c % 2) * HPC
        qkc = np.empty((HPC, D, 2 * S), dtype=np.float16)
        # [s, h, d] -> [h, d, s]
        qkc[:, :, :S] = q[:, b, h0 : h0 + HPC, :].transpose(1, 2, 0)
        qkc[:, :, S:] = k[:, b, h0 : h0 + HPC, :].transpose(1, 2, 0)
        # [s, h, d] -> [h, j, p, d] + ones column -> fp16
        vc = np.ones((HPC, N_SK, 128, VW), dtype=np.float16)
        vc[:, :, :, :D] = (
            v[:, b, h0 : h0 + HPC, :]
            .transpose(1, 0, 2)
            .reshape(HPC, N_SK, 128, D)
            .astype(np.float16)
        )
        in_maps.append({"qk": qkc, "v": vc})
    return in_maps


def run_spmd(in_maps, **kwargs):
    nc = _get_nc()
    return run_bass_kernel_spmd(nc, in_maps, core_ids=list(range(N_CORES)), **kwargs)


def kernel(query_layer, key_layer, value_layer):
    in_maps = _make_in_maps(query_layer, key_layer, value_layer)
    res = run_spmd(in_maps)
    full = np.empty((S, B, H * D), dtype=np.float32)
    for c in range(N_CORES):
        b = c // 2
        h0 = (c % 2) * HPC
        full[:, b, h0 * D : (h0 + HPC) * D] = res.results[c]["out"]
    return full
